# revision 18
# baseline (speedup 1.0000x reference)
"""Trainium2 Bass kernel for nn_LinearAttention (RoPE(Q) @ RoPE(Q)^T @ V).

Algebraic core: no softmax, so out = (QR@QR^T)@V == QR@(QR^T@V) with a
64x64 per-head intermediate. 16 heads / 8 cores = 2 heads per core; the
two heads ride the two 64-wide lanes of the 128x128 PE array.

Layout: t = p*16 + (r*8 + c) (p = SBUF partition, r = range 0/1,
c = chunk-in-range); the host packs/unpacks with this permutation.

v3 schedule:
  - cos (+identity) and sin tables travel on DIFFERENT HWDGE rings and
    their two DMA instructions are hoisted (BIR surgery) before the
    bass-init all-engine barrier, so table data is in flight during the
    last ~us of the fixed preamble. Q lands right behind the tables.
  - RoPE runs ONLY on DVE (GpSimd concurrent with DVE costs DVE ~3x via
    SBUF port contention). Products use an h-broadcast (stride-0) table
    AP; qr is stored [r, x, h, c, kh] so the combines write contiguous
    and the per-chunk matmul lhsT does the (h, x, kh) reorder instead.
  - PE warm-up spam alternates between two PSUM banks (same-bank
    back-to-back matmuls serialize on the fill/drain port) and uses
    regular matmuls (transpose-mode doesn't count for the HAM gate).
  - chunk transposes land 4-per-PSUM-bank; one wide ACT copy evacuates
    each group.
  - phase-3 output blocks: casts alternate DVE/ACT, out-DMAs go on the
    opposite ring, so the block pipeline has no serial engine.
"""

from contextlib import ExitStack

import numpy as np

import concourse.bass as bass
import concourse.mybir as mybir
import concourse.tile as tile
from concourse.bass_utils import run_bass_kernel_spmd
from concourse.vector_clock import ScopedClock

H, T, D = 16, 2048, 64
N_CORES = 8
HPC = H // N_CORES  # heads per core
P = 128
NT = T // P  # 16 t-chunks per head
HD = D // 2
NTABC = NT * HD + P  # cos [r,c,kh]=512 | idt 128
NTABS = NT * HD  # sin 512
F32 = mybir.dt.float32
BF16 = mybir.dt.bfloat16


def _rope_tables():
    inv_freq = 1.0 / (10000.0 ** (np.arange(0, D, 2, dtype=np.float32) / D))
    t = np.arange(T, dtype=np.float32)
    freqs = np.outer(t, inv_freq).astype(np.float32)  # [T, D/2]
    return np.cos(freqs).astype(np.float32), np.sin(freqs).astype(np.float32)


class _SlimTileContext(tile.TileContext):
    """TileContext whose kernel tail uses per-engine drains + a
    sequencer-level (sem-only) barrier instead of the full EVSEM
    butterfly."""

    def _drain_and_barrier(self, tick_clock, wait_clock):
        nc = self.nc
        drain_inst = nc.sync.drain()
        # Engines are ordered by the all_engine_barrier below (engine ops
        # complete in queue order); only ASYNC DMA transfers need the
        # final sem waits, so filter the clock to the DMA lanes. This
        # halves the serial wait-split NoOp chain in the tail.
        import re

        import bass_rust as _br

        gclk = tick_clock.global_clock
        vals = [int(x) for x in re.findall(r"\d+", repr(gclk))]
        dma_clock = _br.VectorClock()
        for idx, name in enumerate(_br.PROC_NAMES):
            if "DMA" in name:
                for _ in range(vals[idx]):
                    dma_clock.advance(idx)
        wait_clock.add_sem_waits(drain_inst.ins, ScopedClock({None: dma_clock}))
        for eng in nc.engines.values():
            if eng.engine != mybir.EngineType.SP:
                eng.drain(fusable=False)
        nc.all_engine_barrier(sem_only=True)
        popped = nc._tile_sem_poison_stack.pop()
        assert popped is self._sem_poison
        nc.clear_and_free_semaphores(list(self.sems.allocated().values()))


def _build_nc():
    nc = bass.Bass()
    TABC = nc.declare_dram_parameter("TABC", [P, NTABC], BF16, isOutput=False)
    TABS = nc.declare_dram_parameter("TABS", [P, NTABS], BF16, isOutput=False)
    # q: [r, x(rot-half), h, c(8), kh(32)] flattened; 8KB/partition so
    # the DMA descriptors hit their efficient size
    QQ = nc.declare_dram_parameter("QQ", [P, 4 * HPC * 8 * HD], BF16, isOutput=False)
    VV = nc.declare_dram_parameter("VV", [P, NT * HPC * D], BF16, isOutput=False)
    OUT = nc.declare_dram_parameter("OUT", [P, T], BF16, isOutput=True)

    hoist_names = []

    with _SlimTileContext(nc) as tc, ExitStack() as ctx:
        singles = ctx.enter_context(tc.tile_pool(name="singles", bufs=1))
        ps_s = ctx.enter_context(tc.tile_pool(name="ps_s", bufs=1, space="PSUM"))
        ps_tp = ctx.enter_context(tc.tile_pool(name="ps_tp", bufs=1, space="PSUM"))

        # --- input DMAs (table DMAs get hoisted pre-barrier) -----------
        tabc_sb = singles.tile([P, NTABC], BF16)
        tabs_sb = singles.tile([P, NTABS], BF16)
        q_sb = singles.tile([P, 2, 2, HPC, 8, HD], BF16)  # [r, x, h, c, kh]
        v_sb = singles.tile([P, NT, HPC, D], BF16)

        # Q rides the SP ring alone (its first range gates RoPE); the
        # tables + V share the ACT ring. Two pieces per tensor: two of
        # the 16 SDMA engines run at ~half rate, so one big transfer's
        # completion semaphore trails ~2.4us behind the bulk of the
        # data; range-sized pieces fire incrementally.
        qv = QQ[:].rearrange("p (r x h c k) -> p r x h c k", r=2, x=2, h=HPC, c=8)
        vv = VV[:].rearrange("p (c h d) -> p c h d", c=NT, h=HPC)
        i1 = nc.sync.dma_start(out=q_sb[:, 0], in_=qv[:, 0])
        i2 = nc.scalar.dma_start(out=tabc_sb, in_=TABC[:])
        hoist_names += [i1.ins.name, i2.ins.name]
        nc.sync.dma_start(out=q_sb[:, 1], in_=qv[:, 1])
        nc.scalar.dma_start(out=tabs_sb, in_=TABS[:])
        nc.scalar.dma_start(out=v_sb[:, 0:8], in_=vv[:, 0:8])
        nc.scalar.dma_start(out=v_sb[:, 8:16], in_=vv[:, 8:16])

        # --- early, dependency-free work -------------------------------
        s2d = singles.tile([P, P], BF16)
        nc.gpsimd.memset(s2d, 0.0)  # off-diagonal stays 0 for phase 3

        cos_t = tabc_sb[:, 0 : NT * HD].rearrange("p (r a c k) -> p r a c k", r=2, a=1, c=8)
        idt = tabc_sb[:, NT * HD :]
        sin_t = tabs_sb.rearrange("p (r a c k) -> p r a c k", r=2, a=1, c=8)

        # idt copy doubles as DVE's TABC-semaphore absorber.
        idt_r = singles.tile([P, P], BF16)
        nc.vector.tensor_copy(out=idt_r, in_=idt)

        # --- tiles -----------------------------------------------------
        cp = singles.tile([P, 2, HPC, 8, HD], BF16)  # [x, h, c, kh]
        sp = singles.tile([P, 2, HPC, 8, HD], BF16)
        qr_r = singles.tile([P, NT, 2, HPC, HD], BF16)  # [c16, x, h, kh]
        qrt_sb = singles.tile([P, NT * P], BF16)
        outT_sb = singles.tile([P, T], BF16)

        s2_ps = ps_s.tile([P, P], F32)
        tp_g = [
            ps_tp.tile([P, 4 * P], F32, tag=f"tp{g}", name=f"tp{g}") for g in range(4)
        ]

        bshape = [P, HPC, 8, HD]
        for r in range(2):
            cosb = cos_t[:, r].to_broadcast(bshape)
            sinb = sin_t[:, r].to_broadcast(bshape)
            # RoPE products on DVE with h-broadcast tables; contiguous IO.
            nc.vector.tensor_mul(cp[:, 0], q_sb[:, r, 0], cosb)
            nc.vector.tensor_mul(sp[:, 1], q_sb[:, r, 1], sinb)
            cs = slice(r * 8, r * 8 + 8)
            # qr_lo = q_lo*cos - q_hi*sin  (chunk-major scatter write)
            nc.vector.tensor_sub(
                qr_r[:, cs, 0].rearrange("p c h k -> p h c k"), cp[:, 0], sp[:, 1]
            )
            nc.vector.tensor_mul(cp[:, 1], q_sb[:, r, 1], cosb)
            nc.vector.tensor_mul(sp[:, 0], q_sb[:, r, 0], sinb)
            # qr_hi = q_hi*cos + q_lo*sin
            nc.vector.tensor_add(
                qr_r[:, cs, 1].rearrange("p c h k -> p h c k"), cp[:, 1], sp[:, 0]
            )

            for j in range(8):
                c = r * 8 + j
                # rows in (h, x, kh) lane order via AP permutation
                # rows in (x, h, kh) order; the chunk slice is fully
                # contiguous, which the weights-AP verifier requires.
                # The head-selection in s2d below matches this row order.
                qr2 = qr_r[:, c].rearrange("p x h k -> p (x h k)")
                v2 = v_sb[:, c].rearrange("p h d -> p (h d)")
                nc.tensor.matmul(
                    s2_ps, lhsT=qr2, rhs=v2, start=(c == 0), stop=(c == NT - 1)
                )
                g, jj = divmod(c, 4)
                nc.tensor.matmul(
                    tp_g[g][:, jj * P : (jj + 1) * P],
                    lhsT=qr2,
                    rhs=idt_r,
                    start=True,
                    stop=True,
                )
                if jj == 3:
                    # one wide ACT copy evacuates 4 transposed chunks
                    nc.scalar.copy(
                        out=qrt_sb[:, g * 4 * P : (g + 1) * 4 * P], in_=tp_g[g]
                    )

        # Head-diagonal blocks of S2 into the (pre-zeroed) phase-3
        # operand. Partition rows are (x, h, kh): head h owns rows
        # {x*64 + h*32 .. +32}; its columns are h*64..h*64+64.
        nc.vector.tensor_copy(out=s2d[0:32, 0:D], in_=s2_ps[0:32, 0:D])
        nc.vector.tensor_copy(out=s2d[32:64, D:], in_=s2_ps[32:64, D:])
        nc.vector.tensor_copy(out=s2d[64:96, 0:D], in_=s2_ps[64:96, 0:D])
        nc.vector.tensor_copy(out=s2d[96:128, D:], in_=s2_ps[96:128, D:])

        # outT blocks: blockdiag(S)^T @ QRT serves both heads at once.
        # Casts alternate DVE/ACT; each block's out-DMA rides the ring
        # whose engine did NOT do the cast.
        for g in range(4):
            # reuse tp bank g (its qrt copy has drained by now): four
            # independent banks let the four matmuls run back-to-back
            o_ps = ps_tp.tile([P, 512], F32, tag=f"tp{g}", name=f"o{g}")
            blk = slice(g * 512, (g + 1) * 512)
            nc.tensor.matmul(
                o_ps, lhsT=s2d, rhs=qrt_sb[:, blk], start=True, stop=True
            )
            lo = slice(g * 512, g * 512 + 256)
            hi = slice(g * 512 + 256, (g + 1) * 512)
            nc.vector.tensor_copy(out=outT_sb[:, lo], in_=o_ps[:, 0:256])
            nc.scalar.copy(out=outT_sb[:, hi], in_=o_ps[:, 256:512])
            eng = nc.sync if g % 2 == 0 else nc.scalar
            eng.dma_start(out=OUT[:, blk], in_=outT_sb[:, blk])

    _split_multi_waits(nc)
    _hoist_input_dmas(nc, hoist_names)
    return nc


def _split_multi_waits(nc):
    """This compiler build rejects instructions carrying more than one
    sync-wait command; split extras into single-wait NoOps placed
    immediately before on the same engine."""
    n = 0
    for f in nc.m.functions:
        for blk in f.blocks:
            new_insts = []
            for inst in blk.instructions:
                si = inst.sync_info
                waits = list(si.on_wait) if si else []
                if len(waits) > 1:
                    for w in waits[:-1]:
                        nop = mybir.InstNoOp(name=f"W-split-{n}", ins=[], outs=[])
                        n += 1
                        nop.engine = inst.engine
                        nop.sync_info = mybir.SyncInfo(on_wait=[w], on_update=[])
                        new_insts.append(nop)
                    inst.sync_info = mybir.SyncInfo(
                        on_wait=[waits[-1]], on_update=list(si.on_update)
                    )
                new_insts.append(inst)
            blk.instructions = new_insts


def _hoist_input_dmas(nc, names):
    """Move the (dependency-free) table DMA issues from the kernel body
    to just before each engine's entry-barrier instruction in `main`, so
    the transfers are in flight during the tail of the fixed preamble.
    The DMA semaphores are runtime-zeroed before the NEFF starts and the
    consumers wait on absolute sem values, so only issue order matters;
    per-engine program order is preserved."""
    names = set(names)
    f = nc.m.functions[0]
    blocks = {b.name: b for b in f.blocks}
    main = blocks["main"]
    moved = []
    for b in f.blocks:
        if b.name == "main":
            continue
        keep = []
        for inst in b.instructions:
            if inst.name in names:
                si = inst.sync_info
                assert not (si and si.on_wait), f"hoisted DMA {inst.name} has waits"
                moved.append(inst)
            else:
                keep.append(inst)
        if len(keep) != len(b.instructions):
            b.instructions = keep
    assert len(moved) == len(names), (len(moved), names)
    new_main = []
    barrier_seen = set()
    for inst in main.instructions:
        if inst.name.startswith("barrier_") and inst.engine not in barrier_seen:
            barrier_seen.add(inst.engine)
            for m in moved:
                if m.engine == inst.engine:
                    new_main.append(m)
        new_main.append(inst)
    main.instructions = new_main


_NC_CACHE = None


def _get_nc():
    global _NC_CACHE
    if _NC_CACHE is None:
        _NC_CACHE = _build_nc()
    return _NC_CACHE


def _pack_inputs(Qs, Vs, cos32, sin32, idt):
    import ml_dtypes

    bf16 = ml_dtypes.bfloat16

    # [T, X] -> [P, NT, X] with t = p*NT + c
    def r(x):
        return x.reshape(P, NT, -1)

    ce = r(cos32).reshape(P, 2, 8, HD)  # [p, r, c, kh]
    se = r(sin32).reshape(P, 2, 8, HD)
    tabc = np.ascontiguousarray(
        np.concatenate([ce.reshape(P, -1), idt], axis=1).astype(bf16)
    )
    tabs = np.ascontiguousarray(se.reshape(P, -1).astype(bf16))

    in_maps = []
    for core in range(N_CORES):
        h0 = core * HPC
        # q[p, r, x, h, c, kh], v[p, c, h, d]
        q = np.empty((P, 2, 2, HPC, 8, HD), np.float32)
        v = np.empty((P, NT, HPC, D), np.float32)
        for h in range(HPC):
            qh = r(Qs[h0 + h]).reshape(P, 2, 8, D)  # [p, r, c, d]
            q[:, :, 0, h] = qh[:, :, :, :HD]
            q[:, :, 1, h] = qh[:, :, :, HD:]
            v[:, :, h] = r(Vs[h0 + h])
        in_maps.append(
            {
                "TABC": tabc,
                "TABS": tabs,
                "QQ": np.ascontiguousarray(q.reshape(P, -1).astype(bf16)),
                "VV": np.ascontiguousarray(v.reshape(P, -1).astype(bf16)),
            }
        )
    return in_maps


def _unpack_out(o):
    # o: [P, T] = outT; rows h*64+j, cols c-major: col = c*128 + f, t = f*16+c
    a = o.reshape(HPC, D, NT, P)  # [h, j, c, f]
    return a.transpose(0, 3, 2, 1).reshape(HPC, T, D)  # [h, t=f*16+c, j]


def run_inner(Q, K, V, trace=False):
    del K  # the module sets KR = QR; K is unused
    Qs = np.asarray(Q, dtype=np.float32)[0]  # [H, T, D]
    Vs = np.asarray(V, dtype=np.float32)[0]
    cos32, sin32 = _rope_tables()
    idt = np.eye(P, dtype=np.float32)
    nc = _get_nc()
    in_maps = _pack_inputs(Qs, Vs, cos32, sin32, idt)
    res = run_bass_kernel_spmd(nc, in_maps, list(range(N_CORES)), trace=trace)
    outs = [_unpack_out(np.asarray(res.results[i]["OUT"])) for i in range(N_CORES)]
    out = np.concatenate(outs, axis=0)[None]  # [1, H, T, D]
    return out.astype(np.float32), res


def kernel(Q, K, V):
    out, _ = run_inner(Q, K, V, trace=False)
    return out
